# revision 38
# baseline (speedup 1.0000x reference)
"""AutoCorrelation (Autoformer-style) sparse attention kernel for 8 trn2 cores.

Math (exact refactoring of the reference):
  mean_corr[b,j] = <((sum_i q[b,i]) @ wq) @ wk.T, keys[b,j]> / (H*L)
  top7 delays d_k + softmax weights w_k over mean_corr
  out[b,l]      = (sum_k w_k * values[b,(l+d_k)%L]) @ (wv@wo)

Sharding: core c handles batch b=c//2, output half h=c%2.

v3 schedule (cost-model driven):
  - bf16 on the wire; input DMAs split over SP / ACT / gpsimd queues with
    q tiles interleaved SP/ACT so the PE qsum chain chases arrivals
  - score path on PE: qsum -> qsumT -> t1 -> t1T -> u -> uT -> s[1,2048]
    (tiny transpose hops via [1,128]x[1,1] matmuls, psum copies on DVE)
  - topk f32 on DVE (Max + MaxIndex); softmax prep on Pool/ACT so DVE goes
    straight from MaxIndex into its mix chunks
  - PE kept at full p-state through the topk window with dummy matmuls
  - mix: DVE 2 chunks (stt), Pool 3 chunks (stt, k0 copies on ACT),
    PE 3 chunks (psum-accumulated matmuls, w_k-scaled identity lhs)
  - finals on PE in mix-readiness order; out = psum copy scaled by 1/sum
    of exps (ACT) then DMA per chunk on SP
"""

import numpy as np
from contextlib import ExitStack

import concourse.bass as bass
import concourse.bacc as bacc
import concourse.mybir as mybir
import concourse.tile as tile
from concourse.bass_utils import run_bass_kernel_spmd

B, L, D, H = 4, 2048, 512, 8
HALF = L // 2
KTOP = 7
EXT = L + HALF
P = 128
FT = D // P
NT = L // P
NCH = HALF // P
F32 = mybir.dt.float32
BF16 = mybir.dt.bfloat16
U32 = mybir.dt.uint32
I32 = mybir.dt.int32
AF = mybir.ActivationFunctionType
ALU = mybir.AluOpType
ENG = mybir.EngineType

N_DUMMY = 4  # PE warm-keepers spanning the Max/MaxIndex window


def _build():
    nc = bacc.Bacc()
    q_d = nc.dram_tensor("q", [L, D], BF16, kind="ExternalInput")
    kt_d = nc.dram_tensor("kt", [D, L], BF16, kind="ExternalInput")
    vt_d = nc.dram_tensor("vt", [D, L], BF16, kind="ExternalInput")
    wq_d = nc.dram_tensor("wq", [D, D], BF16, kind="ExternalInput")
    wkT_d = nc.dram_tensor("wkT", [D, D], BF16, kind="ExternalInput")
    wvT_d = nc.dram_tensor("wvT", [D, D], BF16, kind="ExternalInput")
    wo_d = nc.dram_tensor("wo", [D, D], BF16, kind="ExternalInput")
    cbf_d = nc.dram_tensor("cbf", [P, 257], BF16, kind="ExternalInput")
    cf32_d = nc.dram_tensor("cf32", [1, P], F32, kind="ExternalInput")
    out_d = nc.dram_tensor("out", [HALF, D], BF16, kind="ExternalOutput")

    qdr = q_d.rearrange("(t p) c -> p t c", p=P)
    ktdr = kt_d.rearrange("(f p) l -> p f l", p=P)
    vtdr = vt_d.rearrange("(f p) l -> p f l", p=P)

    with tile.TileContext(nc) as tc, ExitStack() as ctx:
        big = ctx.enter_context(tc.tile_pool(name="big", bufs=1))
        sm = ctx.enter_context(tc.tile_pool(name="sm", bufs=1))
        psp = ctx.enter_context(
            tc.tile_pool(name="psp", bufs=1, space=bass.MemorySpace.PSUM)
        )

        # ---- DMA plan ----------------------------------------------------
        # SP : consts, q pairs 0/2/4/6, wqp, wktp, kT[3]
        # ACT: q pairs 1/3/5/7, kT[0..2]
        # Pool(SWDGE): vt, wvp, wop
        cbf = sm.tile([P, 257], BF16, tag="cbf")
        nc.sync.dma_start(cbf[:], cbf_d[:])
        ident = cbf[:, 0:128]
        ones_col = cbf[:, 128:129]
        one_one = cbf[0:1, 128:129]
        ones_row = cbf[0:1, 129:257]
        cf32 = sm.tile([1, P], F32, tag="cf32")
        nc.sync.dma_start(cf32[:], cf32_d[:])
        ones_row_f = cf32[0:1, 0:128]

        qp = big.tile([P, NT, D], BF16, tag="qp")
        ktp = big.tile([P, FT, L], BF16, tag="ktp")
        for j in range(4):  # SP: tiles 4j,4j+1; ACT: tiles 4j+2,4j+3
            nc.sync.dma_start(
                qp[:, 4 * j : 4 * j + 2, :], qdr[:, 4 * j : 4 * j + 2, :]
            )
            nc.scalar.dma_start(
                qp[:, 4 * j + 2 : 4 * j + 4, :], qdr[:, 4 * j + 2 : 4 * j + 4, :]
            )
        wqp = big.tile([P, FT, D], BF16, tag="wqp")
        nc.sync.dma_start(wqp[:], wq_d.rearrange("(m p) c -> p m c", p=P))
        wktp = big.tile([P, FT, D], BF16, tag="wktp")
        nc.sync.dma_start(wktp[:], wkT_d.rearrange("(m p) c -> p m c", p=P))
        for f in range(3):
            nc.scalar.dma_start(ktp[:, f : f + 1, :], ktdr[:, f : f + 1, :])
        nc.sync.dma_start(ktp[:, 3:4, :], ktdr[:, 3:4, :])

        vt_sb = big.tile([P, FT, EXT], BF16, tag="vt")
        nc.gpsimd.dma_start(vt_sb[:, 0:2, 0:L], vtdr[:, 0:2, :])
        nc.gpsimd.dma_start(vt_sb[:, 2:4, 0:L], vtdr[:, 2:4, :])
        wvp = big.tile([P, FT, D], BF16, tag="wvp")
        nc.sync.dma_start(wvp[:], wvT_d.rearrange("(m p) c -> p m c", p=P))
        wop = big.tile([P, FT, D], BF16, tag="wop")
        nc.scalar.dma_start(wop[:], wo_d.rearrange("(m p) c -> p m c", p=P))
        nc.sync.dma_start(vt_sb[:, :, L:EXT], vt_sb[:, :, 0:HALF])

        # ---- small tiles -------------------------------------------------
        aux2 = sm.tile([P, 32], BF16, tag="aux2")
        qsT = aux2[:, 0:4]
        t1T = aux2[:, 4:8]
        uT = aux2[:, 8:12]
        exbf = aux2[0:1, 16:24]
        wbc = sm.tile([P, KTOP], F32, tag="wbc")
        rse_bc = sm.tile([P, 1], F32, tag="rsebc")
        srow = sm.tile([1, 3 * D], BF16, tag="srow")
        qsum_sb = srow[0:1, 0:D]
        t1_sb = srow[0:1, D : 2 * D]
        u_sb = srow[0:1, 2 * D : 3 * D]
        sfl = sm.tile([1, L], F32, tag="sfl")
        aux3 = sm.tile([1, 48], F32, tag="aux3")
        vals16 = aux3[0:1, 0:16]
        vals8 = aux3[0:1, 16:24]
        exin = aux3[0:1, 24:31]
        se = aux3[0:1, 31:32]
        exf = aux3[0:1, 32:39]
        rse = sm.tile([1, 1], F32, tag="rse")
        idx8 = sm.tile([1, 8], U32, tag="idx8")
        sid = sm.tile([P, KTOP, P], BF16, tag="sid")
        mixs = big.tile([P, NCH, FT, P], BF16, tag="mixs")
        ptmp = sm.tile([P, FT, P], BF16, tag="ptmp")
        ostg = sm.tile([P, 3, D], BF16, tag="ostg")

        # ---- score path on PE -------------------------------------------
        ps_qsum = psp.tile([1, D], F32, tag="a", bufs=2)
        for t in range(NT):
            nc.tensor.matmul(
                ps_qsum[:], ones_col, qp[:, t, :], start=(t == 0), stop=(t == NT - 1)
            )
        def transpose_hop(ps_row, row_sb, colT):
            # four-piece pipelined: per 128-col piece, copy + transpose + col
            # copy, so the next stage's matmul c can chase piece c
            for c in range(FT):
                lo, hi = c * P, (c + 1) * P
                nc.vector.tensor_copy(row_sb[0:1, lo:hi], ps_row[0:1, lo:hi])
                psT = psp.tile([P, 1], F32, tag="a", bufs=2, name=f"psT{c}")
                nc.tensor.matmul(
                    psT[:], row_sb[0:1, lo:hi], one_one,
                    start=True, stop=True, skip_group_check=True,
                )
                nc.vector.tensor_copy(colT[:, c : c + 1], psT[:])

        transpose_hop(ps_qsum, qsum_sb, qsT)
        ps_t1 = psp.tile([1, D], F32, tag="a", bufs=2)
        for mc in range(FT):
            nc.tensor.matmul(
                ps_t1[:], qsT[:, mc : mc + 1], wqp[:, mc, :],
                start=(mc == 0), stop=(mc == FT - 1),
            )
        transpose_hop(ps_t1, t1_sb, t1T)
        ps_u = psp.tile([1, D], F32, tag="a", bufs=2)
        for mc in range(FT):
            nc.tensor.matmul(
                ps_u[:], t1T[:, mc : mc + 1], wktp[:, mc, :],
                start=(mc == 0), stop=(mc == FT - 1),
            )
        transpose_hop(ps_u, u_sb, uT)

        # scores: 4 psum banks, f-major so each pass chases its kT chunk
        ps_s = [
            psp.tile([1, D], F32, tag="s", bufs=4, name=f"ps_s{i}")
            for i in range(FT)
        ]
        for fi, f in enumerate([0, 1, 3, 2]):
            for lc in range(FT):
                nc.tensor.matmul(
                    ps_s[lc][:],
                    uT[:, f : f + 1],
                    ktp[:, f, lc * D : (lc + 1) * D],
                    start=(fi == 0),
                    stop=(fi == FT - 1),
                )
        # copies to flat f32 with 1/(H*L) scale: DVE banks 0,2; Pool 1,3
        for lc in range(FT):
            if lc >= 2:
                nc.scalar.activation(
                    sfl[0:1, lc * D : (lc + 1) * D], ps_s[lc][:], AF.Copy,
                    scale=1.0 / (H * L),
                )
            else:
                nc.vector.tensor_scalar_mul(
                    sfl[0:1, lc * D : (lc + 1) * D], ps_s[lc][:], 1.0 / (H * L)
                )

        # ---- W2 = wv @ wo on PE, ic-outer; floored after the score path --
        w2 = big.tile([P, FT, D], BF16, tag="w2")
        w2ctx = tc.tile_wait_until(0.0145)
        w2ctx.__enter__()
        for ic in range(FT):
            ps_w2 = psp.tile([P, D], F32, tag="b", bufs=2, name=f"ps_w2_{ic}")
            for mc in range(FT):
                nc.tensor.matmul(
                    ps_w2[:],
                    wvp[:, mc, ic * P : (ic + 1) * P],
                    wop[:, mc, :],
                    start=(mc == 0),
                    stop=(mc == FT - 1),
                )
            nc.scalar.copy(w2[:, ic, :], ps_w2[:])
        w2ctx.__exit__(None, None, None)

        # ---- topk on DVE; softmax prep on Pool/ACT ----------------------
        nc.vector.max(vals16[0:1, 0:8], sfl[0:1, 0:HALF])
        nc.vector.max(vals16[0:1, 8:16], sfl[0:1, HALF:L])
        nc.vector.max(vals8, vals16)
        nc.vector.max_index(idx8[:], vals8, sfl[:])

        nc.gpsimd.tensor_scalar_sub(exin, vals8[0:1, 0:KTOP], vals8[0:1, 0:1])
        nc.scalar.activation(exf, exin, AF.Exp, accum_out=se)
        nc.gpsimd.tensor_copy(exbf[0:1, 0:KTOP], exf)
        nc.vector.reciprocal(rse[:], se)

        # broadcasts on PE (emitted before dummies)
        ps_w = psp.tile([P, 8], F32, tag="a", bufs=2)
        nc.tensor.matmul(
            ps_w[:, 0:KTOP], ones_row, exbf[0:1, 0:KTOP], start=True, stop=True
        )
        nc.scalar.copy(wbc[:], ps_w[:, 0:KTOP])
        ps_r = psp.tile([P, 1], F32, tag="a", bufs=2)
        nc.tensor.matmul(ps_r[:], ones_row_f, rse[:], start=True, stop=True)
        nc.scalar.copy(rse_bc[:], ps_r[:])

        # scaled idents on Pool (k=0 uses plain ident)
        for k in range(1, KTOP):
            nc.gpsimd.tensor_scalar_mul(sid[:, k, :], ident, wbc[:, k : k + 1])

        # PE warm-keepers: independent matmuls through the topk window
        ps_dum = psp.tile([P, D], F32, tag="b", bufs=2)
        with tc.tile_wait_until(0.0165):
            for i in range(N_DUMMY):
                nc.tensor.matmul(
                    ps_dum[:], ident, wop[:, 0, :], start=True, stop=True,
                    skip_group_check=True,
                )


        # ---- delay registers --------------------------------------------
        _, dks = nc.values_load_multi_w_load_instructions(
            idx8[0:1, 0:KTOP].bitcast(I32),
            engines=(ENG.DVE, ENG.Pool, ENG.PE, ENG.Activation),
            min_val=0,
            max_val=L - 1,
            skip_runtime_bounds_check=True,
        )

        # ---- mix + finals ------------------------------------------------
        # chunk -> engine: DVE 0,1; Pool 2,3,4; PE 5,6,7
        def emit_pe_mix(ch):
            base = ch * P
            ps_m = psp.tile([P, D], F32, tag="s", bufs=4, name=f"psm{ch}")
            for k in range(KTOP):
                lhs = ident if k == 0 else sid[:, k, :]
                nc.tensor.matmul(
                    ps_m[:],
                    lhs,
                    vt_sb[:, :, bass.ds(dks[k] + base, P)],
                    start=(k == 0),
                    stop=(k == KTOP - 1),
                )
            nc.scalar.copy(mixs[:, ch, :, :], ps_m[:])

        def emit_stt_mix(ch, e, k0_act):
            base = ch * P
            mslice = mixs[:, ch, :, :]
            if k0_act:
                nc.scalar.copy(mslice, vt_sb[:, :, bass.ds(dks[0] + base, P)])
            else:
                nc.gpsimd.tensor_copy(mslice, vt_sb[:, :, bass.ds(dks[0] + base, P)])
            for k in range(1, KTOP):
                e.scalar_tensor_tensor(
                    mslice,
                    vt_sb[:, :, bass.ds(dks[k] + base, P)],
                    wbc[:, k : k + 1],
                    mslice,
                    ALU.mult,
                    ALU.add,
                )

        def emit_pool_mix(ch):
            # Pool dynamic-slice TensorScalarPtr is broken on HW; use
            # dyn-copy (works) + static mul + add, all on Pool
            base = ch * P
            mslice = mixs[:, ch, :, :]
            nc.scalar.copy(mslice, vt_sb[:, :, bass.ds(dks[0] + base, P)])
            for k in range(1, KTOP):
                nc.gpsimd.tensor_copy(ptmp[:], vt_sb[:, :, bass.ds(dks[k] + base, P)])
                nc.gpsimd.tensor_scalar_mul(ptmp[:], ptmp[:], wbc[:, k : k + 1])
                nc.gpsimd.tensor_tensor(mslice, ptmp[:], mslice, ALU.add)

        def emit_final(ch, slot):
            ps_o = psp.tile([P, D], F32, tag="s", bufs=4, name=f"pso{ch}")
            for ft in range(FT):
                nc.tensor.matmul(
                    ps_o[:],
                    mixs[:, ch, ft, :],
                    w2[:, ft, :],
                    start=(ft == 0),
                    stop=(ft == FT - 1),
                )
            stg = ostg[:, slot % 3, :]
            if slot == 7:
                # last chunk: halve copy+DMA so the terminal drain shrinks
                h = D // 2
                nc.scalar.mul(stg[:, 0:h], ps_o[:, 0:h], rse_bc[:])
                nc.sync.dma_start(out_d[ch * P : (ch + 1) * P, 0:h], stg[:, 0:h])
                nc.vector.tensor_scalar_mul(stg[:, h:D], ps_o[:, h:D], rse_bc[:, 0:1])
                nc.scalar.dma_start(out_d[ch * P : (ch + 1) * P, h:D], stg[:, h:D])
            else:
                nc.scalar.mul(stg, ps_o[:], rse_bc[:])
                nc.sync.dma_start(out_d[ch * P : (ch + 1) * P, :], stg)

        emit_pe_mix(5)
        emit_stt_mix(0, nc.vector, k0_act=False)
        emit_pe_mix(6)
        emit_pe_mix(2)
        emit_pool_mix(3)
        emit_pe_mix(7)
        emit_stt_mix(1, nc.vector, k0_act=False)
        emit_stt_mix(4, nc.vector, k0_act=False)
        # benign reader so the BIR verifier accepts the warm-keeper writes
        # (ostg slot 0 is fully overwritten by the real output copy later)
        nc.vector.tensor_copy(ostg[0:1, 0, 0:1], ps_dum[0:1, 0:1])

        # finals in expected mix-readiness order
        for slot, ch in enumerate([5, 6, 2, 7, 0, 3, 1, 4]):
            emit_final(ch, slot)

    return nc


_NC = None
TRACE = False
_LAST_RESULTS = None


def _get_nc():
    global _NC
    if _NC is None:
        _NC = _build()
        _NC.finalize()
    return _NC


def _consts():
    import ml_dtypes

    cbf = np.zeros((P, 257), ml_dtypes.bfloat16)
    cbf[:, 0:128] = np.eye(P, dtype=np.float32)
    cbf[:, 128:257] = 1.0
    cf32 = np.ones((1, P), np.float32)
    return cbf, cf32


def kernel(queries, keys, values, wq, wk, wv, wo):
    import ml_dtypes

    nc = _get_nc()
    bf = ml_dtypes.bfloat16
    wq_b = np.ascontiguousarray(wq, dtype=bf)
    wkT_b = np.ascontiguousarray(wk.T, dtype=bf)
    wvT_b = np.ascontiguousarray(wv.T, dtype=bf)
    wo_b = np.ascontiguousarray(wo, dtype=bf)
    cbf, cf32 = _consts()
    in_maps = []
    for c in range(8):
        b, h = divmod(c, 2)
        vrot = np.roll(values[b], -h * HALF, axis=0)
        in_maps.append(
            {
                "q": np.ascontiguousarray(queries[b], dtype=bf),
                "kt": np.ascontiguousarray(keys[b].T, dtype=bf),
                "vt": np.ascontiguousarray(vrot.T, dtype=bf),
                "wq": wq_b,
                "wkT": wkT_b,
                "wvT": wvT_b,
                "wo": wo_b,
                "cbf": cbf,
                "cf32": cf32,
            }
        )
    global _LAST_RESULTS
    res = run_bass_kernel_spmd(nc, in_maps, list(range(8)), trace=TRACE)
    _LAST_RESULTS = res
    out = np.empty((B, L, D), np.float32)
    for c in range(8):
        b, h = divmod(c, 2)
        out[b, h * HALF : (h + 1) * HALF] = np.asarray(res.results[c]["out"], np.float32)
    return out


# revision 39
# speedup vs baseline: 1.0104x; 1.0104x over previous
"""AutoCorrelation (Autoformer-style) sparse attention kernel for 8 trn2 cores.

Math (exact refactoring of the reference):
  mean_corr[b,j] = <((sum_i q[b,i]) @ wq) @ wk.T, keys[b,j]> / (H*L)
  top7 delays d_k + softmax weights w_k over mean_corr
  out[b,l]      = (sum_k w_k * values[b,(l+d_k)%L]) @ (wv@wo)

Sharding: core c handles batch b=c//2, output half h=c%2.

v3 schedule (cost-model driven):
  - bf16 on the wire; input DMAs split over SP / ACT / gpsimd queues with
    q tiles interleaved SP/ACT so the PE qsum chain chases arrivals
  - score path on PE: qsum -> qsumT -> t1 -> t1T -> u -> uT -> s[1,2048]
    (tiny transpose hops via [1,128]x[1,1] matmuls, psum copies on DVE)
  - topk f32 on DVE (Max + MaxIndex); softmax prep on Pool/ACT so DVE goes
    straight from MaxIndex into its mix chunks
  - PE kept at full p-state through the topk window with dummy matmuls
  - mix: DVE 2 chunks (stt), Pool 3 chunks (stt, k0 copies on ACT),
    PE 3 chunks (psum-accumulated matmuls, w_k-scaled identity lhs)
  - finals on PE in mix-readiness order; out = psum copy scaled by 1/sum
    of exps (ACT) then DMA per chunk on SP
"""

import numpy as np
from contextlib import ExitStack

import concourse.bass as bass
import concourse.bacc as bacc
import concourse.mybir as mybir
import concourse.tile as tile
from concourse.bass_utils import run_bass_kernel_spmd

B, L, D, H = 4, 2048, 512, 8
HALF = L // 2
KTOP = 7
EXT = L + HALF
P = 128
FT = D // P
NT = L // P
NCH = HALF // P
F32 = mybir.dt.float32
BF16 = mybir.dt.bfloat16
U32 = mybir.dt.uint32
I32 = mybir.dt.int32
AF = mybir.ActivationFunctionType
ALU = mybir.AluOpType
ENG = mybir.EngineType

N_DUMMY = 4  # PE warm-keepers spanning the Max/MaxIndex window


def _build():
    nc = bacc.Bacc()
    q_d = nc.dram_tensor("q", [L, D], BF16, kind="ExternalInput")
    kt_d = nc.dram_tensor("kt", [D, L], BF16, kind="ExternalInput")
    vt_d = nc.dram_tensor("vt", [D, L], BF16, kind="ExternalInput")
    wq_d = nc.dram_tensor("wq", [D, D], BF16, kind="ExternalInput")
    wkT_d = nc.dram_tensor("wkT", [D, D], BF16, kind="ExternalInput")
    wvT_d = nc.dram_tensor("wvT", [D, D], BF16, kind="ExternalInput")
    wo_d = nc.dram_tensor("wo", [D, D], BF16, kind="ExternalInput")
    cbf_d = nc.dram_tensor("cbf", [P, 257], BF16, kind="ExternalInput")
    cf32_d = nc.dram_tensor("cf32", [1, P], F32, kind="ExternalInput")
    out_d = nc.dram_tensor("out", [HALF, D], BF16, kind="ExternalOutput")

    qdr = q_d.rearrange("(t p) c -> p t c", p=P)
    ktdr = kt_d.rearrange("(f p) l -> p f l", p=P)
    vtdr = vt_d.rearrange("(f p) l -> p f l", p=P)

    with tile.TileContext(nc) as tc, ExitStack() as ctx:
        big = ctx.enter_context(tc.tile_pool(name="big", bufs=1))
        sm = ctx.enter_context(tc.tile_pool(name="sm", bufs=1))
        psp = ctx.enter_context(
            tc.tile_pool(name="psp", bufs=1, space=bass.MemorySpace.PSUM)
        )

        # ---- DMA plan ----------------------------------------------------
        # SP : consts, q pairs 0/2/4/6, wqp, wktp, kT[3]
        # ACT: q pairs 1/3/5/7, kT[0..2]
        # Pool(SWDGE): vt, wvp, wop
        cbf = sm.tile([P, 257], BF16, tag="cbf")
        nc.sync.dma_start(cbf[:], cbf_d[:])
        ident = cbf[:, 0:128]
        ones_col = cbf[:, 128:129]
        one_one = cbf[0:1, 128:129]
        ones_row = cbf[0:1, 129:257]
        cf32 = sm.tile([1, P], F32, tag="cf32")
        nc.sync.dma_start(cf32[:], cf32_d[:])
        ones_row_f = cf32[0:1, 0:128]

        qp = big.tile([P, NT, D], BF16, tag="qp")
        ktp = big.tile([P, FT, L], BF16, tag="ktp")
        for j in range(4):  # SP: tiles 4j,4j+1; ACT: tiles 4j+2,4j+3
            nc.sync.dma_start(
                qp[:, 4 * j : 4 * j + 2, :], qdr[:, 4 * j : 4 * j + 2, :]
            )
            nc.scalar.dma_start(
                qp[:, 4 * j + 2 : 4 * j + 4, :], qdr[:, 4 * j + 2 : 4 * j + 4, :]
            )
        wqp = big.tile([P, FT, D], BF16, tag="wqp")
        nc.sync.dma_start(wqp[:], wq_d.rearrange("(m p) c -> p m c", p=P))
        wktp = big.tile([P, FT, D], BF16, tag="wktp")
        nc.sync.dma_start(wktp[:], wkT_d.rearrange("(m p) c -> p m c", p=P))
        for f in range(3):
            nc.scalar.dma_start(ktp[:, f : f + 1, :], ktdr[:, f : f + 1, :])
        nc.sync.dma_start(ktp[:, 3:4, :], ktdr[:, 3:4, :])

        vt_sb = big.tile([P, FT, EXT], BF16, tag="vt")
        nc.gpsimd.dma_start(vt_sb[:, 0:2, 0:L], vtdr[:, 0:2, :])
        nc.gpsimd.dma_start(vt_sb[:, 2:4, 0:L], vtdr[:, 2:4, :])
        wvp = big.tile([P, FT, D], BF16, tag="wvp")
        nc.sync.dma_start(wvp[:], wvT_d.rearrange("(m p) c -> p m c", p=P))
        wop = big.tile([P, FT, D], BF16, tag="wop")
        nc.scalar.dma_start(wop[:], wo_d.rearrange("(m p) c -> p m c", p=P))
        nc.sync.dma_start(vt_sb[:, :, L:EXT], vt_sb[:, :, 0:HALF])

        # ---- small tiles -------------------------------------------------
        aux2 = sm.tile([P, 32], BF16, tag="aux2")
        qsT = aux2[:, 0:4]
        t1T = aux2[:, 4:8]
        uT = aux2[:, 8:12]
        exbf = aux2[0:1, 16:24]
        wbc = sm.tile([P, KTOP], F32, tag="wbc")
        rse_bc = sm.tile([P, 1], F32, tag="rsebc")
        srow = sm.tile([1, 3 * D], BF16, tag="srow")
        qsum_sb = srow[0:1, 0:D]
        t1_sb = srow[0:1, D : 2 * D]
        u_sb = srow[0:1, 2 * D : 3 * D]
        sfl = sm.tile([1, L], F32, tag="sfl")
        aux3 = sm.tile([1, 48], F32, tag="aux3")
        vals16 = aux3[0:1, 0:16]
        vals8 = aux3[0:1, 16:24]
        exin = aux3[0:1, 24:31]
        se = aux3[0:1, 31:32]
        exf = aux3[0:1, 32:39]
        rse = sm.tile([1, 1], F32, tag="rse")
        idx8 = sm.tile([1, 8], U32, tag="idx8")
        sid = sm.tile([P, KTOP, P], BF16, tag="sid")
        mixs = big.tile([P, NCH, FT, P], BF16, tag="mixs")
        ptmp = sm.tile([P, FT, P], BF16, tag="ptmp")
        ostg = sm.tile([P, 3, D], BF16, tag="ostg")

        # ---- score path on PE -------------------------------------------
        ps_qsum = psp.tile([1, D], F32, tag="a", bufs=2)
        for t in range(NT):
            nc.tensor.matmul(
                ps_qsum[:], ones_col, qp[:, t, :], start=(t == 0), stop=(t == NT - 1)
            )
        def transpose_hop(ps_row, row_sb, colT):
            # four-piece pipelined: per 128-col piece, copy + transpose + col
            # copy, so the next stage's matmul c can chase piece c
            for c in range(FT):
                lo, hi = c * P, (c + 1) * P
                nc.vector.tensor_copy(row_sb[0:1, lo:hi], ps_row[0:1, lo:hi])
                psT = psp.tile([P, 1], F32, tag="a", bufs=2, name=f"psT{c}")
                nc.tensor.matmul(
                    psT[:], row_sb[0:1, lo:hi], one_one,
                    start=True, stop=True, skip_group_check=True,
                )
                nc.vector.tensor_copy(colT[:, c : c + 1], psT[:])

        transpose_hop(ps_qsum, qsum_sb, qsT)
        ps_t1 = psp.tile([1, D], F32, tag="a", bufs=2)
        for mc in range(FT):
            nc.tensor.matmul(
                ps_t1[:], qsT[:, mc : mc + 1], wqp[:, mc, :],
                start=(mc == 0), stop=(mc == FT - 1),
            )
        transpose_hop(ps_t1, t1_sb, t1T)
        ps_u = psp.tile([1, D], F32, tag="a", bufs=2)
        for mc in range(FT):
            nc.tensor.matmul(
                ps_u[:], t1T[:, mc : mc + 1], wktp[:, mc, :],
                start=(mc == 0), stop=(mc == FT - 1),
            )
        transpose_hop(ps_u, u_sb, uT)

        # scores: 4 psum banks, f-major so each pass chases its kT chunk
        ps_s = [
            psp.tile([1, D], F32, tag="s", bufs=4, name=f"ps_s{i}")
            for i in range(FT)
        ]
        for fi, f in enumerate([0, 1, 3, 2]):
            for lc in range(FT):
                nc.tensor.matmul(
                    ps_s[lc][:],
                    uT[:, f : f + 1],
                    ktp[:, f, lc * D : (lc + 1) * D],
                    start=(fi == 0),
                    stop=(fi == FT - 1),
                )
        # copies to flat f32 with 1/(H*L) scale: DVE banks 0,2; Pool 1,3
        for lc in range(FT):
            if lc >= 2:
                nc.scalar.activation(
                    sfl[0:1, lc * D : (lc + 1) * D], ps_s[lc][:], AF.Copy,
                    scale=1.0 / (H * L),
                )
            else:
                nc.vector.tensor_scalar_mul(
                    sfl[0:1, lc * D : (lc + 1) * D], ps_s[lc][:], 1.0 / (H * L)
                )

        # ---- W2 = wv @ wo on PE, ic-outer; floored after the score path --
        w2 = big.tile([P, FT, D], BF16, tag="w2")
        w2ctx = tc.tile_wait_until(0.0145)
        w2ctx.__enter__()
        for ic in range(FT):
            ps_w2 = psp.tile([P, D], F32, tag="b", bufs=2, name=f"ps_w2_{ic}")
            for mc in range(FT):
                nc.tensor.matmul(
                    ps_w2[:],
                    wvp[:, mc, ic * P : (ic + 1) * P],
                    wop[:, mc, :],
                    start=(mc == 0),
                    stop=(mc == FT - 1),
                )
            nc.scalar.copy(w2[:, ic, :], ps_w2[:])
        w2ctx.__exit__(None, None, None)

        # ---- topk on DVE; softmax prep on Pool/ACT ----------------------
        nc.vector.max(vals16[0:1, 0:8], sfl[0:1, 0:HALF])
        nc.vector.max(vals16[0:1, 8:16], sfl[0:1, HALF:L])
        nc.vector.max(vals8, vals16)
        nc.vector.max_index(idx8[:], vals8, sfl[:])

        nc.gpsimd.tensor_scalar_sub(exin, vals8[0:1, 0:KTOP], vals8[0:1, 0:1])
        nc.scalar.activation(exf, exin, AF.Exp, accum_out=se)
        nc.gpsimd.tensor_copy(exbf[0:1, 0:KTOP], exf)
        nc.vector.reciprocal(rse[:], se)

        # broadcasts on PE (emitted before dummies)
        ps_w = psp.tile([P, 8], F32, tag="a", bufs=2)
        nc.tensor.matmul(
            ps_w[:, 0:KTOP], ones_row, exbf[0:1, 0:KTOP], start=True, stop=True
        )
        nc.scalar.copy(wbc[:], ps_w[:, 0:KTOP])
        ps_r = psp.tile([P, 1], F32, tag="a", bufs=2)
        nc.tensor.matmul(ps_r[:], ones_row_f, rse[:], start=True, stop=True)
        nc.scalar.copy(rse_bc[:], ps_r[:])

        # scaled idents on Pool (k=0 uses plain ident)
        for k in range(1, KTOP):
            nc.gpsimd.tensor_scalar_mul(sid[:, k, :], ident, wbc[:, k : k + 1])

        # PE warm-keepers: independent matmuls through the topk window
        ps_dum = psp.tile([P, D], F32, tag="b", bufs=2)
        with tc.tile_wait_until(0.0165):
            for i in range(N_DUMMY):
                nc.tensor.matmul(
                    ps_dum[:], ident, wop[:, 0, :], start=True, stop=True,
                    skip_group_check=True,
                )


        # ---- delay registers --------------------------------------------
        _, dks = nc.values_load_multi_w_load_instructions(
            idx8[0:1, 0:KTOP].bitcast(I32),
            engines=(ENG.DVE, ENG.Pool, ENG.PE, ENG.Activation),
            min_val=0,
            max_val=L - 1,
            skip_runtime_bounds_check=True,
        )

        # ---- mix + finals ------------------------------------------------
        # chunk -> engine: DVE 0,1; Pool 2,3,4; PE 5,6,7
        def emit_pe_mix(ch):
            base = ch * P
            ps_m = psp.tile([P, D], F32, tag="s", bufs=4, name=f"psm{ch}")
            for k in range(KTOP):
                lhs = ident if k == 0 else sid[:, k, :]
                nc.tensor.matmul(
                    ps_m[:],
                    lhs,
                    vt_sb[:, :, bass.ds(dks[k] + base, P)],
                    start=(k == 0),
                    stop=(k == KTOP - 1),
                )
            nc.scalar.copy(mixs[:, ch, :, :], ps_m[:])

        def emit_stt_mix(ch, e, k0_act):
            base = ch * P
            mslice = mixs[:, ch, :, :]
            if k0_act:
                nc.scalar.copy(mslice, vt_sb[:, :, bass.ds(dks[0] + base, P)])
            else:
                nc.gpsimd.tensor_copy(mslice, vt_sb[:, :, bass.ds(dks[0] + base, P)])
            for k in range(1, KTOP):
                e.scalar_tensor_tensor(
                    mslice,
                    vt_sb[:, :, bass.ds(dks[k] + base, P)],
                    wbc[:, k : k + 1],
                    mslice,
                    ALU.mult,
                    ALU.add,
                )

        def emit_pool_mix(ch):
            # Pool dynamic-slice TensorScalarPtr is broken on HW; use
            # dyn-copy (works) + static mul + add, all on Pool
            base = ch * P
            mslice = mixs[:, ch, :, :]
            nc.scalar.copy(mslice, vt_sb[:, :, bass.ds(dks[0] + base, P)])
            for k in range(1, KTOP):
                nc.gpsimd.tensor_copy(ptmp[:], vt_sb[:, :, bass.ds(dks[k] + base, P)])
                nc.gpsimd.tensor_scalar_mul(ptmp[:], ptmp[:], wbc[:, k : k + 1])
                nc.gpsimd.tensor_tensor(mslice, ptmp[:], mslice, ALU.add)

        def emit_final(ch, slot):
            ps_o = psp.tile([P, D], F32, tag="s", bufs=4, name=f"pso{ch}")
            for ft in range(FT):
                nc.tensor.matmul(
                    ps_o[:],
                    mixs[:, ch, ft, :],
                    w2[:, ft, :],
                    start=(ft == 0),
                    stop=(ft == FT - 1),
                )
            stg = ostg[:, slot % 3, :]
            nc.scalar.mul(stg, ps_o[:], rse_bc[:])
            nc.sync.dma_start(out_d[ch * P : (ch + 1) * P, :], stg)

        emit_pe_mix(5)
        emit_stt_mix(0, nc.vector, k0_act=False)
        emit_pe_mix(6)
        emit_pe_mix(2)
        emit_pool_mix(3)
        emit_pe_mix(7)
        emit_stt_mix(1, nc.vector, k0_act=False)
        emit_stt_mix(4, nc.vector, k0_act=False)
        # benign reader so the BIR verifier accepts the warm-keeper writes
        # (ostg slot 0 is fully overwritten by the real output copy later)
        nc.vector.tensor_copy(ostg[0:1, 0, 0:1], ps_dum[0:1, 0:1])

        # finals in expected mix-readiness order
        for slot, ch in enumerate([5, 6, 2, 7, 0, 3, 1, 4]):
            emit_final(ch, slot)

    return nc


_NC = None
TRACE = False
_LAST_RESULTS = None


def _get_nc():
    global _NC
    if _NC is None:
        _NC = _build()
        _NC.finalize()
    return _NC


def _consts():
    import ml_dtypes

    cbf = np.zeros((P, 257), ml_dtypes.bfloat16)
    cbf[:, 0:128] = np.eye(P, dtype=np.float32)
    cbf[:, 128:257] = 1.0
    cf32 = np.ones((1, P), np.float32)
    return cbf, cf32


def kernel(queries, keys, values, wq, wk, wv, wo):
    import ml_dtypes

    nc = _get_nc()
    bf = ml_dtypes.bfloat16
    wq_b = np.ascontiguousarray(wq, dtype=bf)
    wkT_b = np.ascontiguousarray(wk.T, dtype=bf)
    wvT_b = np.ascontiguousarray(wv.T, dtype=bf)
    wo_b = np.ascontiguousarray(wo, dtype=bf)
    cbf, cf32 = _consts()
    in_maps = []
    for c in range(8):
        b, h = divmod(c, 2)
        vrot = np.roll(values[b], -h * HALF, axis=0)
        in_maps.append(
            {
                "q": np.ascontiguousarray(queries[b], dtype=bf),
                "kt": np.ascontiguousarray(keys[b].T, dtype=bf),
                "vt": np.ascontiguousarray(vrot.T, dtype=bf),
                "wq": wq_b,
                "wkT": wkT_b,
                "wvT": wvT_b,
                "wo": wo_b,
                "cbf": cbf,
                "cf32": cf32,
            }
        )
    global _LAST_RESULTS
    res = run_bass_kernel_spmd(nc, in_maps, list(range(8)), trace=TRACE)
    _LAST_RESULTS = res
    out = np.empty((B, L, D), np.float32)
    for c in range(8):
        b, h = divmod(c, 2)
        out[b, h * HALF : (h + 1) * HALF] = np.asarray(res.results[c]["out"], np.float32)
    return out
